# revision 33
# baseline (speedup 1.0000x reference)
"""Trainium2 Bass kernel for nn_ConvexReLUCNN.

Math (identical multilinear form as the reference, reordered):
    reference:  U = unfold(x,3); A = U.G^T (54 GFLOP); out = A.(v-w)
    here:       T[(p,z), (i,w)] = sum_{q,m} pd[m,(c,p,q,o)] * Gpad[m, i, w-q]
                (q-shifts realized as rhs column offsets, summed in PSUM)
                Wmat[z, (h,w)]  = sum_p T[(p,z), (h+2-p, w)]   (3 adds)
                out = x_flat @ Wmat                            (~0.13 GFLOP)

Distribution: sharded by image row band. Core i owns output-image rows
h in [8i, 8i+8) (all channels, all widths, ALL batches):
  - x shard (pre-transposed on host to [chw, b], bf16): (1536, 512)
  - G shard: patch-grid rows [8i-2, 8i+8), each row zero-padded 62->64
    with 2 extra leading zero cols so every q/row shift stays in
    bounds, bf16: (512, 642)
  - pd = v - w computed on host, permuted to [q, p*32 + c*10 + o], bf16
Each core computes partial out^T (10, 512) over its chw band; the host
sums the 8 partials and transposes - no device collectives needed.

All layout work (transpose/interleave/zero-pad/cast) is host-side
sharding; all contractions run on device.
"""

import numpy as np
from contextlib import ExitStack

import ml_dtypes

import concourse.bass as bass
import concourse.mybir as mybir
import concourse.tile as tile
from concourse import bacc
from concourse.bass_utils import run_bass_kernel_spmd
from concourse.masks import make_identity

N_CORES = 8
B_FULL = 512
C_CH, H, W = 3, 64, 64
HB = H // N_CORES           # 8 image rows per core
M = 512                     # num_neurons
O = 10
Ho = Wo = 62
IW = HB + 2                 # 10 patch-grid rows feeding one band
GPW = 2 + IW * W            # 642 padded G columns
Z = 32                      # padded (c,o) block per p: 3*10 -> 32
PZ = 3 * Z                  # 96 T rows (p, z)

F32 = mybir.dt.float32
F16 = mybir.dt.float16
BF16 = mybir.dt.bfloat16
NP_BF16 = ml_dtypes.bfloat16

_NC = None


def _build():
    nc = bacc.Bacc("TRN2", target_bir_lowering=False, debug=False,
                   num_devices=N_CORES)
    # host supplies all operands pre-interleaved for straight 128-partition
    # DMAs with large per-partition-contiguous descriptors: dim0 = partition.
    x_d = nc.dram_tensor("x", [128, 2, 12, 256], BF16,
                         kind="ExternalInput").ap()
    g_d = nc.dram_tensor("G", [128, 4, GPW], BF16, kind="ExternalInput").ap()
    pd_d = nc.dram_tensor("pd", [128, 4, 3, PZ], BF16,
                          kind="ExternalInput").ap()
    o_d = nc.dram_tensor("out", [O, B_FULL], F32, kind="ExternalOutput").ap()

    with tile.TileContext(nc) as tc, ExitStack() as ctx:
        const = ctx.enter_context(tc.tile_pool(name="const", bufs=1))
        big = ctx.enter_context(tc.tile_pool(name="big", bufs=1))
        psC = ctx.enter_context(tc.tile_pool(name="psC", bufs=1, space="PSUM"))
        psT = ctx.enter_context(tc.tile_pool(name="psT", bufs=2, space="PSUM"))
        psF = ctx.enter_context(tc.tile_pool(name="psF", bufs=2, space="PSUM"))
        stage = ctx.enter_context(tc.tile_pool(name="stage", bufs=2))

        # ---- loads first (nothing may delay DMA issue) -------------------
        gs = big.tile([128, 4, GPW], BF16, tag="gs")
        pds = big.tile([128, 4, 3, PZ], BF16, tag="pds")
        xs = big.tile([128, 2, 12, 256], BF16, tag="xs")
        # each dma_start costs ~0.7us of serial descriptor generation per
        # HWDGE engine, and descriptors hit the 16 queues in generation
        # order. sync feeds the G tiles in the exact order C consumes
        # them; scalar feeds pd and the two x batch-halves in parallel so
        # everything is queued by ~t+2us and the aggregate drains at full
        # HBM rate.
        nc.scalar.dma_start(pds[:], pd_d)
        nc.scalar.dma_start(xs[:, 0], x_d[:, 0])
        nc.scalar.dma_start(xs[:, 1], x_d[:, 1])
        for m in range(4):
            nc.sync.dma_start(gs[:, m], g_d[:, m])

        idq = const.tile([Z, Z], F16)
        make_identity(nc, idq[:])

        # ---- PE warmup: the tensor clock ramps to full speed only after
        # ~3us of sustained activity (and stays up across short idles), so
        # burn dummy matmuls while the DMAs land - C then streams at full
        # rate from its first row.
        dw = const.tile([128, 64], F16, tag="dw")
        dx = const.tile([128, 512], F16, tag="dx")
        nc.gpsimd.memset(dw[:], 0.0)
        nc.gpsimd.memset(dx[:], 0.0)
        psW = ctx.enter_context(tc.tile_pool(name="psW", bufs=1, space="PSUM"))
        wps = psW.tile([64, 512], F32, tag="wps")
        for _ in range(6):
            nc.tensor.matmul(wps[:], dw[:], dx[:], start=True, stop=True)

        # ---- T[(p,z), (i,w)] = sum_{q,m} pd.T @ Gpad[(2-q) shifted] ------
        # one [96, 640] PSUM accumulation over all 12 (q, m) pairs; the two
        # column pieces keep each matmul inside one PSUM bank.
        # m outer so each G m-tile is consumed for ~3 matmul pairs of
        # compute while the next tile's DMA lands; both column pieces stay
        # adjacent per (m, q) so the second LDWEIGHTS of the shared lhsT is
        # warm.
        ps = psC.tile([PZ, 644], F32, tag="psC")
        for m in range(4):
            for q in range(3):
                lhs = pds[:, m, q, :]
                first, last = (m == 0 and q == 0), (m == 3 and q == 2)
                nc.tensor.matmul(ps[:, 0:512], lhs,
                                 gs[:, m, 2 - q:2 - q + 512],
                                 start=first, stop=last)
                nc.tensor.matmul(ps[:, 512:640], lhs,
                                 gs[:, m, 2 - q + 512:2 - q + 640],
                                 start=first, stop=last)

        # keep the PE clock up through the fold idle with more dummies
        # (they run between C's last matmul and the first transpose).
        for _ in range(6):
            nc.tensor.matmul(wps[:], dw[:], dx[:], start=True, stop=True)

        # ---- p-fold: Wacc[z, (h,w)] = sum_p T[(p,z), (h+2-p, w)] ---------
        # serial on vector, and finals only start after ALL vector work:
        # vector activity alongside final matmuls halves their stream rate
        # (SBUF port contention), so keep the phases disjoint.
        Wacc = big.tile([Z, HB * W], F16, tag="Wacc")
        nc.vector.tensor_copy(Wacc[:], ps[64:96, 0:512])
        nc.vector.tensor_add(Wacc[:], Wacc[:], ps[32:64, 64:576])
        nc.vector.tensor_add(Wacc[:], Wacc[:], ps[0:32, 128:640])

        # ---- transpose Wacc chunks -> WsB[hw_in_chunk, j, z] -------------
        WsB = big.tile([128, 4, Z], BF16, tag="WsB")
        for j in range(4):
            pst = psT.tile([128, Z], F16, tag="psT")
            nc.tensor.transpose(pst[:], Wacc[:, 128 * j:128 * (j + 1)],
                                idq[:])
            nc.vector.tensor_copy(WsB[:, j, :], pst[:])

        # ---- final: partial out^T[o, b] over this core's 12 chw chunks ---
        # two batch halves so the first half's store overlaps the second
        # half's matmuls.
        for hi in (0, 1):
            pf = psF.tile([O, 256], F32, tag="pf")
            n = 0
            for j in range(4):
                for c in range(3):
                    nc.tensor.matmul(pf[:, :],
                                     WsB[:, j, c * O:(c + 1) * O],
                                     xs[:, hi, c * 4 + j, :],
                                     start=(n == 0), stop=(n == 11))
                    n += 1
            obuf = stage.tile([O, 256], F32, tag="obuf")
            nc.vector.tensor_copy(obuf[:], pf[:])
            nc.sync.dma_start(o_d[:, 256 * hi:256 * (hi + 1)], obuf[:])
    nc.compile()
    return nc


def _get_nc():
    global _NC
    if _NC is None:
        _NC = _build()
    return _NC


def _shard_inputs(inputs):
    x = np.ascontiguousarray(inputs["x"], dtype=np.float32)   # (512,3,64,64)
    G = np.ascontiguousarray(inputs["G"], dtype=np.float32)   # (512,3844)
    v = np.ascontiguousarray(inputs["v"], dtype=np.float32)
    w = np.ascontiguousarray(inputs["w"], dtype=np.float32)

    # pd permuted to [q, p*32 + c*10 + o], m-interleaved
    pdfull = (v - w).reshape(M, 3, 3, 3, O)       # [m, c, p, q, o]
    pdq = np.zeros((M, 3, 3, Z), np.float32)      # [m, q, p, z]
    for c in range(C_CH):
        pdq[:, :, :, c * O:(c + 1) * O] = pdfull[:, c].transpose(0, 2, 1, 3)
    pdh = np.ascontiguousarray(
        pdq.reshape(4, 128, 3, PZ).transpose(1, 0, 2, 3)).astype(NP_BF16)

    Gim = G.reshape(M, Ho, Wo)
    in_maps = []
    for i in range(N_CORES):
        h0 = HB * i
        # x band, transposed to [chw, b] then 128-partition interleaved
        xb = x[:, :, h0:h0 + HB, :].transpose(1, 2, 3, 0)     # (c,h,w,b)
        xh = np.ascontiguousarray(
            xb.reshape(12, 128, 2, 256).transpose(1, 2, 0, 3)).astype(
                NP_BF16)
        # G shard with halo: rows padded 62->64 plus 2 leading zero cols
        gsh = np.zeros((M, IW, Wo), np.float32)
        lo, hi = h0 - 2, h0 + HB
        clo, chi = max(lo, 0), min(hi, Ho)
        gsh[:, clo - lo:chi - lo, :] = Gim[:, clo:chi, :]
        gp = np.zeros((M, GPW), np.float32)
        for r in range(IW):
            gp[:, 2 + r * W:2 + r * W + Wo] = gsh[:, r, :]
        gh = np.ascontiguousarray(
            gp.reshape(4, 128, GPW).transpose(1, 0, 2)).astype(NP_BF16)
        in_maps.append({"x": xh, "G": gh, "pd": pdh})
    return in_maps


def _run(inputs, trace=False, **kw):
    nc = _get_nc()
    in_maps = _shard_inputs(inputs)
    res = run_bass_kernel_spmd(nc, in_maps, list(range(N_CORES)),
                               trace=trace, **kw)
    out = np.zeros((O, B_FULL), np.float64)
    for i in range(N_CORES):
        out += res.results[i]["out"].astype(np.float64)
    return np.ascontiguousarray(out.T).astype(np.float32), res


def kernel(**inputs) -> np.ndarray:
    return _run(inputs)[0]


# revision 35
# speedup vs baseline: 1.1714x; 1.1714x over previous
"""Trainium2 Bass kernel for nn_ConvexReLUCNN.

Math (identical multilinear form as the reference, reordered):
    reference:  U = unfold(x,3); A = U.G^T (54 GFLOP); out = A.(v-w)
    here:       T[(p,z), (i,w)] = sum_{q,m} pd[m,(c,p,q,o)] * Gpad[m, i, w-q]
                (q-shifts realized as rhs column offsets, summed in PSUM)
                Wmat[z, (h,w)]  = sum_p T[(p,z), (h+2-p, w)]   (3 adds)
                out = x_flat @ Wmat                            (~0.13 GFLOP)

Distribution: sharded by image row band. Core i owns output-image rows
h in [8i, 8i+8) (all channels, all widths, ALL batches):
  - x shard (pre-transposed on host to [chw, b], bf16): (1536, 512)
  - G shard: patch-grid rows [8i-2, 8i+8), each row zero-padded 62->64
    with 2 extra leading zero cols so every q/row shift stays in
    bounds, bf16: (512, 642)
  - pd = v - w computed on host, permuted to [q, p*32 + c*10 + o], bf16
Each core computes partial out^T (10, 512) over its chw band; the host
sums the 8 partials and transposes - no device collectives needed.

All layout work (transpose/interleave/zero-pad/cast) is host-side
sharding; all contractions run on device.
"""

import numpy as np
from contextlib import ExitStack

import ml_dtypes

import concourse.bass as bass
import concourse.mybir as mybir
import concourse.tile as tile
from concourse import bacc
from concourse.bass_utils import run_bass_kernel_spmd
from concourse.masks import make_identity

N_CORES = 8
B_FULL = 512
C_CH, H, W = 3, 64, 64
HB = H // N_CORES           # 8 image rows per core
M = 512                     # num_neurons
O = 10
Ho = Wo = 62
IW = HB + 2                 # 10 patch-grid rows feeding one band
GPW = 2 + IW * W            # 642 padded G columns
Z = 32                      # padded (c,o) block per p: 3*10 -> 32
PZ = 3 * Z                  # 96 T rows (p, z)

F32 = mybir.dt.float32
F16 = mybir.dt.float16
BF16 = mybir.dt.bfloat16
NP_BF16 = ml_dtypes.bfloat16

_NC = None


def _build():
    nc = bacc.Bacc("TRN2", target_bir_lowering=False, debug=False,
                   num_devices=N_CORES)
    # host supplies all operands pre-interleaved for straight 128-partition
    # DMAs with large per-partition-contiguous descriptors: dim0 = partition.
    x_d = nc.dram_tensor("x", [128, 2, 12, 256], BF16,
                         kind="ExternalInput").ap()
    g_d = nc.dram_tensor("G", [128, 4, GPW], BF16, kind="ExternalInput").ap()
    pd_d = nc.dram_tensor("pd", [128, 4, 3, PZ], BF16,
                          kind="ExternalInput").ap()
    o_d = nc.dram_tensor("out", [O, B_FULL], F32, kind="ExternalOutput").ap()

    with tile.TileContext(nc) as tc, ExitStack() as ctx:
        const = ctx.enter_context(tc.tile_pool(name="const", bufs=1))
        big = ctx.enter_context(tc.tile_pool(name="big", bufs=1))
        psC = ctx.enter_context(tc.tile_pool(name="psC", bufs=1, space="PSUM"))
        psT = ctx.enter_context(tc.tile_pool(name="psT", bufs=2, space="PSUM"))
        psF = ctx.enter_context(tc.tile_pool(name="psF", bufs=2, space="PSUM"))
        stage = ctx.enter_context(tc.tile_pool(name="stage", bufs=2))

        # ---- loads first (nothing may delay DMA issue) -------------------
        gs = big.tile([128, 4, GPW], BF16, tag="gs")
        pds = big.tile([128, 4, 3, PZ], BF16, tag="pds")
        xs = big.tile([128, 2, 12, 256], BF16, tag="xs")
        # each dma_start costs ~0.7us of serial descriptor generation per
        # HWDGE engine, and descriptors hit the 16 queues in generation
        # order. sync feeds the G tiles in the exact order C consumes
        # them; scalar feeds pd and the two x batch-halves in parallel so
        # everything is queued by ~t+2us and the aggregate drains at full
        # HBM rate.
        nc.scalar.dma_start(pds[:], pd_d)
        for m in range(4):
            nc.sync.dma_start(gs[:, m], g_d[:, m])
        nc.sync.dma_start(xs[:, 0], x_d[:, 0])
        nc.sync.dma_start(xs[:, 1], x_d[:, 1])

        idq = const.tile([Z, Z], F16)
        make_identity(nc, idq[:])

        # ---- PE warmup: the tensor clock ramps to full speed only after
        # ~3us of sustained activity (and stays up across short idles), so
        # burn dummy matmuls while the DMAs land - C then streams at full
        # rate from its first row.
        dw = const.tile([128, 64], F16, tag="dw")
        dx = const.tile([128, 512], F16, tag="dx")
        nc.gpsimd.memset(dw[:], 0.0)
        nc.gpsimd.memset(dx[:], 0.0)
        psW = ctx.enter_context(tc.tile_pool(name="psW", bufs=1, space="PSUM"))
        wps = psW.tile([64, 512], F32, tag="wps")
        for _ in range(6):
            nc.tensor.matmul(wps[:], dw[:], dx[:], start=True, stop=True)

        # ---- T[(p,z), (i,w)] = sum_{q,m} pd.T @ Gpad[(2-q) shifted] ------
        # one [96, 640] PSUM accumulation over all 12 (q, m) pairs; the two
        # column pieces keep each matmul inside one PSUM bank.
        # m outer so each G m-tile is consumed for ~3 matmul pairs of
        # compute while the next tile's DMA lands; both column pieces stay
        # adjacent per (m, q) so the second LDWEIGHTS of the shared lhsT is
        # warm.
        ps = psC.tile([PZ, 644], F32, tag="psC")
        for m in range(4):
            for q in range(3):
                lhs = pds[:, m, q, :]
                first, last = (m == 0 and q == 0), (m == 3 and q == 2)
                nc.tensor.matmul(ps[:, 0:512], lhs,
                                 gs[:, m, 2 - q:2 - q + 512],
                                 start=first, stop=last)
                nc.tensor.matmul(ps[:, 512:640], lhs,
                                 gs[:, m, 2 - q + 512:2 - q + 640],
                                 start=first, stop=last)

        # ---- p-fold: Wacc[z, (h,w)] = sum_p T[(p,z), (h+2-p, w)] ---------
        # serial on vector, and finals only start after ALL vector work:
        # vector activity alongside final matmuls halves their stream rate
        # (SBUF port contention), so keep the phases disjoint.
        Wacc = big.tile([Z, HB * W], F16, tag="Wacc")
        nc.vector.tensor_copy(Wacc[:], ps[64:96, 0:512])
        nc.vector.tensor_add(Wacc[:], Wacc[:], ps[32:64, 64:576])
        nc.vector.tensor_add(Wacc[:], Wacc[:], ps[0:32, 128:640])

        # ---- transpose Wacc chunks -> WsB[hw_in_chunk, j, z] -------------
        WsB = big.tile([128, 4, Z], BF16, tag="WsB")
        for j in range(4):
            pst = psT.tile([128, Z], F16, tag="psT")
            nc.tensor.transpose(pst[:], Wacc[:, 128 * j:128 * (j + 1)],
                                idq[:])
            nc.vector.tensor_copy(WsB[:, j, :], pst[:])

        # ---- final: partial out^T[o, b] over this core's 12 chw chunks ---
        # two batch halves so the first half's store overlaps the second
        # half's matmuls.
        for hi in (0, 1):
            pf = psF.tile([O, 256], F32, tag="pf")
            n = 0
            for j in range(4):
                for c in range(3):
                    nc.tensor.matmul(pf[:, :],
                                     WsB[:, j, c * O:(c + 1) * O],
                                     xs[:, hi, c * 4 + j, :],
                                     start=(n == 0), stop=(n == 11))
                    n += 1
            obuf = stage.tile([O, 256], F32, tag="obuf")
            nc.vector.tensor_copy(obuf[:], pf[:])
            nc.sync.dma_start(o_d[:, 256 * hi:256 * (hi + 1)], obuf[:])
    nc.compile()
    return nc


def _get_nc():
    global _NC
    if _NC is None:
        _NC = _build()
    return _NC


def _shard_inputs(inputs):
    x = np.ascontiguousarray(inputs["x"], dtype=np.float32)   # (512,3,64,64)
    G = np.ascontiguousarray(inputs["G"], dtype=np.float32)   # (512,3844)
    v = np.ascontiguousarray(inputs["v"], dtype=np.float32)
    w = np.ascontiguousarray(inputs["w"], dtype=np.float32)

    # pd permuted to [q, p*32 + c*10 + o], m-interleaved
    pdfull = (v - w).reshape(M, 3, 3, 3, O)       # [m, c, p, q, o]
    pdq = np.zeros((M, 3, 3, Z), np.float32)      # [m, q, p, z]
    for c in range(C_CH):
        pdq[:, :, :, c * O:(c + 1) * O] = pdfull[:, c].transpose(0, 2, 1, 3)
    pdh = np.ascontiguousarray(
        pdq.reshape(4, 128, 3, PZ).transpose(1, 0, 2, 3)).astype(NP_BF16)

    Gim = G.reshape(M, Ho, Wo)
    in_maps = []
    for i in range(N_CORES):
        h0 = HB * i
        # x band, transposed to [chw, b] then 128-partition interleaved
        xb = x[:, :, h0:h0 + HB, :].transpose(1, 2, 3, 0)     # (c,h,w,b)
        xh = np.ascontiguousarray(
            xb.reshape(12, 128, 2, 256).transpose(1, 2, 0, 3)).astype(
                NP_BF16)
        # G shard with halo: rows padded 62->64 plus 2 leading zero cols
        gsh = np.zeros((M, IW, Wo), np.float32)
        lo, hi = h0 - 2, h0 + HB
        clo, chi = max(lo, 0), min(hi, Ho)
        gsh[:, clo - lo:chi - lo, :] = Gim[:, clo:chi, :]
        gp = np.zeros((M, GPW), np.float32)
        for r in range(IW):
            gp[:, 2 + r * W:2 + r * W + Wo] = gsh[:, r, :]
        gh = np.ascontiguousarray(
            gp.reshape(4, 128, GPW).transpose(1, 0, 2)).astype(NP_BF16)
        in_maps.append({"x": xh, "G": gh, "pd": pdh})
    return in_maps


def _run(inputs, trace=False, **kw):
    nc = _get_nc()
    in_maps = _shard_inputs(inputs)
    res = run_bass_kernel_spmd(nc, in_maps, list(range(N_CORES)),
                               trace=trace, **kw)
    out = np.zeros((O, B_FULL), np.float64)
    for i in range(N_CORES):
        out += res.results[i]["out"].astype(np.float64)
    return np.ascontiguousarray(out.T).astype(np.float32), res


def kernel(**inputs) -> np.ndarray:
    return _run(inputs)[0]


# revision 36
# speedup vs baseline: 1.2162x; 1.0383x over previous
"""Trainium2 Bass kernel for nn_ConvexReLUCNN.

Math (identical multilinear form as the reference, reordered):
    reference:  U = unfold(x,3); A = U.G^T (54 GFLOP); out = A.(v-w)
    here:       T[(p,z), (i,w)] = sum_{q,m} pd[m,(c,p,q,o)] * Gpad[m, i, w-q]
                (q-shifts realized as rhs column offsets, summed in PSUM)
                Wmat[z, (h,w)]  = sum_p T[(p,z), (h+2-p, w)]   (3 adds)
                out = x_flat @ Wmat                            (~0.13 GFLOP)

Distribution: sharded by image row band. Core i owns output-image rows
h in [8i, 8i+8) (all channels, all widths, ALL batches):
  - x shard (pre-transposed on host to [chw, b], bf16): (1536, 512)
  - G shard: patch-grid rows [8i-2, 8i+8), each row zero-padded 62->64
    with 2 extra leading zero cols so every q/row shift stays in
    bounds, bf16: (512, 642)
  - pd = v - w computed on host, permuted to [q, p*32 + c*10 + o], bf16
Each core computes partial out^T (10, 512) over its chw band; the host
sums the 8 partials and transposes - no device collectives needed.

All layout work (transpose/interleave/zero-pad/cast) is host-side
sharding; all contractions run on device.
"""

import numpy as np
from contextlib import ExitStack

import ml_dtypes

import concourse.bass as bass
import concourse.mybir as mybir
import concourse.tile as tile
from concourse import bacc
from concourse.bass_utils import run_bass_kernel_spmd
from concourse.masks import make_identity

N_CORES = 8
B_FULL = 512
C_CH, H, W = 3, 64, 64
HB = H // N_CORES           # 8 image rows per core
M = 512                     # num_neurons
O = 10
Ho = Wo = 62
IW = HB + 2                 # 10 patch-grid rows feeding one band
GPW = 2 + IW * W            # 642 padded G columns
Z = 32                      # padded (c,o) block per p: 3*10 -> 32
PZ = 3 * Z                  # 96 T rows (p, z)

F32 = mybir.dt.float32
F16 = mybir.dt.float16
BF16 = mybir.dt.bfloat16
NP_BF16 = ml_dtypes.bfloat16

_NC = None


def _build():
    nc = bacc.Bacc("TRN2", target_bir_lowering=False, debug=False,
                   num_devices=N_CORES)
    # host supplies all operands pre-interleaved for straight 128-partition
    # DMAs with large per-partition-contiguous descriptors: dim0 = partition.
    x_d = nc.dram_tensor("x", [128, 2, 12, 256], BF16,
                         kind="ExternalInput").ap()
    g_d = nc.dram_tensor("G", [128, 4, GPW], BF16, kind="ExternalInput").ap()
    pd_d = nc.dram_tensor("pd", [128, 4, 3, PZ], BF16,
                          kind="ExternalInput").ap()
    o_d = nc.dram_tensor("out", [O, B_FULL], F32, kind="ExternalOutput").ap()

    with tile.TileContext(nc) as tc, ExitStack() as ctx:
        const = ctx.enter_context(tc.tile_pool(name="const", bufs=1))
        big = ctx.enter_context(tc.tile_pool(name="big", bufs=1))
        psC = ctx.enter_context(tc.tile_pool(name="psC", bufs=1, space="PSUM"))
        psT = ctx.enter_context(tc.tile_pool(name="psT", bufs=2, space="PSUM"))
        psF = ctx.enter_context(tc.tile_pool(name="psF", bufs=2, space="PSUM"))
        stage = ctx.enter_context(tc.tile_pool(name="stage", bufs=2))

        # ---- loads first (nothing may delay DMA issue) -------------------
        gs = big.tile([128, 4, GPW], BF16, tag="gs")
        pds = big.tile([128, 4, 3, PZ], BF16, tag="pds")
        xs = big.tile([128, 2, 12, 256], BF16, tag="xs")
        # each dma_start costs ~0.7us of serial descriptor generation per
        # HWDGE engine, and descriptors hit the 16 queues in generation
        # order. sync feeds the G tiles in the exact order C consumes
        # them; scalar feeds pd and the two x batch-halves in parallel so
        # everything is queued by ~t+2us and the aggregate drains at full
        # HBM rate.
        nc.scalar.dma_start(pds[:], pd_d)
        for m in range(4):
            nc.sync.dma_start(gs[:, m], g_d[:, m])
        nc.sync.dma_start(xs[:, 0], x_d[:, 0])
        nc.sync.dma_start(xs[:, 1], x_d[:, 1])

        idq = const.tile([Z, Z], F16)
        make_identity(nc, idq[:])

        # ---- PE warmup: the tensor clock ramps to full speed only after
        # ~3us of sustained activity (and stays up across short idles), so
        # burn dummy matmuls while the DMAs land - C then streams at full
        # rate from its first row.
        dw = const.tile([128, 64], F16, tag="dw")
        dx = const.tile([128, 512], F16, tag="dx")
        nc.gpsimd.memset(dw[:], 0.0)
        nc.gpsimd.memset(dx[:], 0.0)
        psW = ctx.enter_context(tc.tile_pool(name="psW", bufs=1, space="PSUM"))
        wps = psW.tile([64, 512], F32, tag="wps")
        for _ in range(6):
            nc.tensor.matmul(wps[:], dw[:], dx[:], start=True, stop=True)

        # ---- T[(p,z), (i,w)] = sum_{q,m} pd.T @ Gpad[(2-q) shifted] ------
        # one [96, 640] PSUM accumulation over all 12 (q, m) pairs; the two
        # column pieces keep each matmul inside one PSUM bank.
        # m outer so each G m-tile is consumed for ~3 matmul pairs of
        # compute while the next tile's DMA lands; both column pieces stay
        # adjacent per (m, q) so the second LDWEIGHTS of the shared lhsT is
        # warm.
        ps = psC.tile([PZ, 644], F32, tag="psC")
        for m in range(4):
            for q in range(3):
                lhs = pds[:, m, q, :]
                first, last = (m == 0 and q == 0), (m == 3 and q == 2)
                nc.tensor.matmul(ps[:, 0:512], lhs,
                                 gs[:, m, 2 - q:2 - q + 512],
                                 start=first, stop=last)
                nc.tensor.matmul(ps[:, 512:640], lhs,
                                 gs[:, m, 2 - q + 512:2 - q + 640],
                                 start=first, stop=last)

        # keep the PE clock up through the fold (PE would idle ~2.2us and
        # drop to half rate for the first finals otherwise); these finish
        # before the fold ends, so they delay nothing.
        for _ in range(5):
            nc.tensor.matmul(wps[:], dw[:], dx[:], start=True, stop=True)

        # ---- p-fold: Wacc[z, (h,w)] = sum_p T[(p,z), (h+2-p, w)] ---------
        # serial on vector, and finals only start after ALL vector work:
        # vector activity alongside final matmuls halves their stream rate
        # (SBUF port contention), so keep the phases disjoint.
        Wacc = big.tile([Z, HB * W], F16, tag="Wacc")
        nc.vector.tensor_copy(Wacc[:], ps[64:96, 0:512])
        nc.vector.tensor_add(Wacc[:], Wacc[:], ps[32:64, 64:576])
        nc.vector.tensor_add(Wacc[:], Wacc[:], ps[0:32, 128:640])

        # ---- transpose Wacc chunks -> WsB[hw_in_chunk, j, z] -------------
        WsB = big.tile([128, 4, Z], BF16, tag="WsB")
        for j in range(4):
            pst = psT.tile([128, Z], F16, tag="psT")
            nc.tensor.transpose(pst[:], Wacc[:, 128 * j:128 * (j + 1)],
                                idq[:])
            nc.vector.tensor_copy(WsB[:, j, :], pst[:])

        # ---- final: partial out^T[o, b] over this core's 12 chw chunks ---
        # two batch halves so the first half's store overlaps the second
        # half's matmuls.
        for hi in (0, 1):
            pf = psF.tile([O, 256], F32, tag="pf")
            n = 0
            for j in range(4):
                for c in range(3):
                    nc.tensor.matmul(pf[:, :],
                                     WsB[:, j, c * O:(c + 1) * O],
                                     xs[:, hi, c * 4 + j, :],
                                     start=(n == 0), stop=(n == 11))
                    n += 1
            obuf = stage.tile([O, 256], F32, tag="obuf")
            nc.vector.tensor_copy(obuf[:], pf[:])
            nc.sync.dma_start(o_d[:, 256 * hi:256 * (hi + 1)], obuf[:])
    nc.compile()
    return nc


def _get_nc():
    global _NC
    if _NC is None:
        _NC = _build()
    return _NC


def _shard_inputs(inputs):
    x = np.ascontiguousarray(inputs["x"], dtype=np.float32)   # (512,3,64,64)
    G = np.ascontiguousarray(inputs["G"], dtype=np.float32)   # (512,3844)
    v = np.ascontiguousarray(inputs["v"], dtype=np.float32)
    w = np.ascontiguousarray(inputs["w"], dtype=np.float32)

    # pd permuted to [q, p*32 + c*10 + o], m-interleaved
    pdfull = (v - w).reshape(M, 3, 3, 3, O)       # [m, c, p, q, o]
    pdq = np.zeros((M, 3, 3, Z), np.float32)      # [m, q, p, z]
    for c in range(C_CH):
        pdq[:, :, :, c * O:(c + 1) * O] = pdfull[:, c].transpose(0, 2, 1, 3)
    pdh = np.ascontiguousarray(
        pdq.reshape(4, 128, 3, PZ).transpose(1, 0, 2, 3)).astype(NP_BF16)

    Gim = G.reshape(M, Ho, Wo)
    in_maps = []
    for i in range(N_CORES):
        h0 = HB * i
        # x band, transposed to [chw, b] then 128-partition interleaved
        xb = x[:, :, h0:h0 + HB, :].transpose(1, 2, 3, 0)     # (c,h,w,b)
        xh = np.ascontiguousarray(
            xb.reshape(12, 128, 2, 256).transpose(1, 2, 0, 3)).astype(
                NP_BF16)
        # G shard with halo: rows padded 62->64 plus 2 leading zero cols
        gsh = np.zeros((M, IW, Wo), np.float32)
        lo, hi = h0 - 2, h0 + HB
        clo, chi = max(lo, 0), min(hi, Ho)
        gsh[:, clo - lo:chi - lo, :] = Gim[:, clo:chi, :]
        gp = np.zeros((M, GPW), np.float32)
        for r in range(IW):
            gp[:, 2 + r * W:2 + r * W + Wo] = gsh[:, r, :]
        gh = np.ascontiguousarray(
            gp.reshape(4, 128, GPW).transpose(1, 0, 2)).astype(NP_BF16)
        in_maps.append({"x": xh, "G": gh, "pd": pdh})
    return in_maps


def _run(inputs, trace=False, **kw):
    nc = _get_nc()
    in_maps = _shard_inputs(inputs)
    res = run_bass_kernel_spmd(nc, in_maps, list(range(N_CORES)),
                               trace=trace, **kw)
    out = np.zeros((O, B_FULL), np.float64)
    for i in range(N_CORES):
        out += res.results[i]["out"].astype(np.float64)
    return np.ascontiguousarray(out.T).astype(np.float32), res


def kernel(**inputs) -> np.ndarray:
    return _run(inputs)[0]
